# revision 1
# baseline (speedup 1.0000x reference)
"""Multi-head causal attention (B=4, S=2048, D=1024, H=16) on 8 TRN2 NeuronCores.

Sharding: core c -> (batch b = c//2, head-group g = c%2). Each core computes
8 heads for one batch: QKV projection (tensor-parallel column slice), causal
softmax attention, and a row-parallel slice of the output projection. The two
cores of a batch produce partial outputs that the host sums; biases that
commute with the attention (v bias, out bias) are folded into a single
host-side vector add.

All matmuls run in float32r (11-bit-mantissa fp32, full PE rate). Device
layout notes:
 - q/k are produced transposed: qT[p] = [128 partitions (2 heads x 64 hd), S],
   so scoresT[j, i] = kT.T @ qT has keys j on partitions; the two heads of a
   pair run as concurrent row-tiled matmuls (tile_position (0,0)/(64,0)).
 - v is produced in [s, dv] layout with an interleaved ones column per head
   ([v_h | 1], width 65) so attn@v also yields the softmax denominator row.
 - causal handling: off-diagonal j-tiles are full 512-wide matmuls; diagonal
   j-tiles compute only the valid right part (width 512-128r), mask the one
   triangular 128x128 boundary block additively, and zero-fill the invalid
   left part of the exp tile on GpSimd. The key (padding) mask enters as the
   per-partition bias of the exp activation (0 or -1e30 per key).
 - softmax denominators: reciprocal on DVE, broadcast across partitions on
   GpSimd, one multiply into the normalized attention tile.
"""

import numpy as np
from contextlib import ExitStack

B, S, D, H = 4, 2048, 1024, 16
HD = D // H          # 64
HPC = H // 2         # 8 heads per core
DV = HPC * HD        # 512 v-dims per core
N_CORES = 8
SB = 512             # i-tile width (matmul N)
NSB = S // SB        # 4
NJT = S // 128       # 16 j-tiles

_CACHE = {}


def _build_module():
    import os
    KREP = int(os.environ.get("KREP", "1"))
    SPSB = int(os.environ.get("SPSB", "6"))
    APSB = int(os.environ.get("APSB", "2"))
    EPB = int(os.environ.get("EPB", "6"))
    XPB = int(os.environ.get("XPB", "10"))
    PS1B = int(os.environ.get("PS1B", "6"))
    NPB = int(os.environ.get("NPB", "8"))
    YPB = int(os.environ.get("YPB", "8"))
    import concourse.bacc as bacc
    import concourse.mybir as mybir
    import concourse.tile as tile
    from concourse._compat import get_trn_type

    F32 = mybir.dt.float32
    F32R = mybir.dt.float32r
    EXP = mybir.ActivationFunctionType.Exp

    nc = bacc.Bacc(get_trn_type() or "TRN2", target_bir_lowering=False, debug=False)

    # ---- DRAM parameters (per core) ----
    xT = nc.declare_dram_parameter("xT", [D, S], F32R, isOutput=False)        # x[b].T
    wq = nc.declare_dram_parameter("wq", [D, DV], F32R, isOutput=False)       # (W_q,g / 8).T
    wk = nc.declare_dram_parameter("wk", [D, DV], F32R, isOutput=False)       # W_k,g.T
    wv = nc.declare_dram_parameter("wv", [D, DV], F32R, isOutput=False)       # W_v,g.T
    ow = nc.declare_dram_parameter("ow", [DV, D], F32R, isOutput=False)       # W_out[:, g].T
    bq = nc.declare_dram_parameter("bq", [DV, 1], F32, isOutput=False)        # q bias / 8
    bk = nc.declare_dram_parameter("bk", [DV, 1], F32, isOutput=False)
    kb = nc.declare_dram_parameter("kb", [S, 1], F32, isOutput=False)         # key-mask bias
    y = nc.declare_dram_parameter("y", [S, D], F32, isOutput=True)            # partial output

    with tile.TileContext(nc) as tc, ExitStack() as octx:
        # ---- persistent SBUF ----
        pers = octx.enter_context(tc.tile_pool(name="pers", bufs=1))
        qT = [pers.tile([128, S], F32R, tag=f"qT{p}", name=f"qT{p}") for p in range(4)]
        kT = [pers.tile([128, S], F32R, tag=f"kT{p}", name=f"kT{p}") for p in range(4)]
        vx = [pers.tile([128, HPC * 65], F32R, tag=f"vx{j}", name=f"vx{j}") for j in range(NJT)]
        anT = [pers.tile([128, S], F32R, tag=f"anT{p}", name=f"anT{p}") for p in range(4)]
        bq_t = pers.tile([128, 4], F32, tag="bq")
        bk_t = pers.tile([128, 4], F32, tag="bk")
        kb_t = pers.tile([128, NJT], F32, tag="kb")
        cmt = pers.tile([128, 128], F32, tag="cmt")   # triangular boundary mask

        nc.sync.dma_start(bq_t[:], bq[:].squeeze(1).rearrange("(t p) -> p t", p=128))
        nc.sync.dma_start(bk_t[:], bk[:].squeeze(1).rearrange("(t p) -> p t", p=128))
        nc.sync.dma_start(kb_t[:], kb[:].squeeze(1).rearrange("(t p) -> p t", p=128))

        # keep (0) iff c - pj >= 0, else -1e30  (boundary block: col c = local
        # query offset, partition pj = key offset within the diagonal block)
        nc.vector.memset(cmt[:], 0.0)
        nc.gpsimd.affine_select(
            out=cmt[:], in_=cmt[:], compare_op=mybir.AluOpType.is_ge,
            fill=-1e30, base=0, pattern=[[1, 128]], channel_multiplier=-1,
        )

        # ones columns of vx tiles (col 64 of each 65-wide head slot)
        for j in range(NJT):
            ones_view = vx[j][:].bitcast(F32).rearrange("p (h c) -> p h c", c=65)[:, :, 64:65]
            nc.vector.memset(ones_view, 1.0)

        for _rep in range(KREP):
            # ---- phase 1: qkv projection ----
            with ExitStack() as ctx1:
                wpool = ctx1.enter_context(tc.tile_pool(name="wpool", bufs=1))
                wq_t = [wpool.tile([128, DV], F32R, tag=f"wq{d}", name=f"wq{d}") for d in range(8)]
                wk_t = [wpool.tile([128, DV], F32R, tag=f"wk{d}", name=f"wk{d}") for d in range(8)]
                wv_t = [wpool.tile([128, DV], F32R, tag=f"wv{d}", name=f"wv{d}") for d in range(8)]
                for d in range(8):
                    nc.sync.dma_start(wq_t[d][:], wq[128 * d:128 * d + 128, :])
                    nc.sync.dma_start(wk_t[d][:], wk[128 * d:128 * d + 128, :])
                    nc.sync.dma_start(wv_t[d][:], wv[128 * d:128 * d + 128, :])

                xpool = ctx1.enter_context(tc.tile_pool(name="xpool", bufs=XPB))
                ps1 = ctx1.enter_context(tc.tile_pool(name="ps1", bufs=PS1B, space="PSUM"))

                for sblk in range(NSB):
                    ssl = slice(SB * sblk, SB * sblk + SB)
                    xt = []
                    for d in range(8):
                        t = xpool.tile([128, SB], F32R, tag="xt")
                        nc.sync.dma_start(t[:], xT[128 * d:128 * d + 128, ssl])
                        xt.append(t)
                    for wt, bt, dst in ((wq_t, bq_t, qT), (wk_t, bk_t, kT)):
                        for o in range(4):
                            osl = slice(128 * o, 128 * o + 128)
                            ps = ps1.tile([128, SB], F32, tag="ps")
                            for d in range(8):
                                nc.tensor.matmul(ps[:], wt[d][:, osl], xt[d][:],
                                                 start=(d == 0), stop=(d == 7))
                            nc.vector.tensor_scalar_add(dst[o][:, ssl], ps[:], bt[:, o:o + 1])
                    for ssub in range(4):
                        jt = 4 * sblk + ssub
                        ps = ps1.tile([128, SB], F32, tag="ps")
                        for d in range(8):
                            nc.tensor.matmul(ps[:], xt[d][:, 128 * ssub:128 * ssub + 128],
                                             wv_t[d][:], start=(d == 0), stop=(d == 7))
                        dst = vx[jt][:].rearrange("p (h c) -> p h c", c=65)[:, :, 0:64]
                        src = ps[:].rearrange("p (h c) -> p h c", c=64)
                        nc.vector.tensor_copy(dst, src)

            # ---- phases 2-4 share the preloaded output-projection weights ----
            with ExitStack() as ctx23:
                opool = ctx23.enter_context(tc.tile_pool(name="opool", bufs=1))
                ow_t = [opool.tile([128, SB], F32R, tag=f"ow{i}", name=f"ow{i}") for i in range(8)]
                for p in range(4):
                    for ot in range(2):
                        nc.sync.dma_start(ow_t[2 * p + ot][:],
                                          ow[128 * p:128 * p + 128, SB * ot:SB * ot + SB])

                # ---- phases 2+3+4: attention fused with output projection,
                # i-tile-major so each i-window's projection overlaps the next
                # window's attention. Phase-4 psum groups share the "sps" slots.
                with ExitStack() as ctx2:
                    sps = ctx2.enter_context(tc.tile_pool(name="sps", bufs=SPSB, space="PSUM"))
                    aps = ctx2.enter_context(tc.tile_pool(name="aps", bufs=APSB, space="PSUM"))
                    epool = ctx2.enter_context(tc.tile_pool(name="epool", bufs=EPB))
                    npool = ctx2.enter_context(tc.tile_pool(name="npool", bufs=NPB))
                    ypool = ctx2.enter_context(tc.tile_pool(name="ypool", bufs=YPB))

                    for it in range(NSB):
                        i0 = SB * it
                        for p in range(4):
                            pa = aps.tile([65, SB], F32, tag="aps")
                            pb = aps.tile([65, SB], F32, tag="aps")
                            njt = 4 * it + 4
                            for jt in range(njt):
                                jsl = slice(128 * jt, 128 * jt + 128)
                                r = jt - 4 * it          # negative: off-diagonal
                                c0 = 128 * r if r > 0 else 0  # first valid col in i-window
                                w = SB - c0
                                sA = sps.tile([128, w], F32, tag="sps")
                                sB = sps.tile([128, w], F32, tag="sps")
                                nc.tensor.matmul(sA[:], kT[p][0:64, jsl],
                                                 qT[p][0:64, i0 + c0:i0 + SB],
                                                 start=True, stop=True)
                                nc.tensor.matmul(sB[:], kT[p][64:128, jsl],
                                                 qT[p][64:128, i0 + c0:i0 + SB],
                                                 start=True, stop=True)
                                if r >= 0:  # triangular boundary block at local cols 0:128
                                    nc.vector.tensor_add(sA[:, 0:128], sA[:, 0:128], cmt[:])
                                    nc.vector.tensor_add(sB[:, 0:128], sB[:, 0:128], cmt[:])
                                eA = epool.tile([128, SB], F32R, tag="e")
                                eB = epool.tile([128, SB], F32R, tag="e")
                                if c0 > 0:
                                    nc.gpsimd.memset(eA[:, 0:c0].bitcast(F32), 0.0)
                                    nc.gpsimd.memset(eB[:, 0:c0].bitcast(F32), 0.0)
                                nc.scalar.activation(eA[:, c0:SB], sA[:], EXP,
                                                     bias=kb_t[:, jt:jt + 1])
                                nc.scalar.activation(eB[:, c0:SB], sB[:], EXP,
                                                     bias=kb_t[:, jt:jt + 1])
                                va = vx[jt][:, 65 * (2 * p):65 * (2 * p) + 65]
                                vb = vx[jt][:, 65 * (2 * p + 1):65 * (2 * p + 1) + 65]
                                nc.tensor.matmul(pa[:], va, eA[:],
                                                 start=(jt == 0), stop=(jt == njt - 1))
                                nc.tensor.matmul(pb[:], vb, eB[:],
                                                 start=(jt == 0), stop=(jt == njt - 1))
                            for ps_t, half in ((pa, 0), (pb, 1)):
                                rec = npool.tile([1, SB], F32, tag="rec")
                                nc.vector.reciprocal(rec[:], ps_t[64:65, :])
                                rb = npool.tile([64, SB], F32, tag="rb")
                                nc.gpsimd.partition_broadcast(rb[:], rec[:])
                                out = anT[p][64 * half:64 * half + 64, i0:i0 + SB]
                                nc.vector.tensor_mul(out, ps_t[0:64, :], rb[:])
                        # project this i-window (s rows i0:i0+SB)
                        for st in range(4 * it, 4 * it + 4):
                            ssl = slice(128 * st, 128 * st + 128)
                            for ot in range(2):
                                ps = sps.tile([128, SB], F32, tag="sps")
                                for p in range(4):
                                    nc.tensor.matmul(ps[:], anT[p][:, ssl], ow_t[2 * p + ot][:],
                                                     start=(p == 0), stop=(p == 3))
                                yt = ypool.tile([128, SB], F32, tag="yt")
                                nc.vector.tensor_copy(yt[:], ps[:])
                                nc.sync.dma_start(y[ssl, SB * ot:SB * ot + SB], yt[:])

    nc.compile()
    return nc


def _get_module():
    if "nc" not in _CACHE:
        _CACHE["nc"] = _build_module()
    return _CACHE["nc"]


def _host_prep(x, mask, qkv_w, qkv_b, out_w):
    """Per-core input maps."""
    scale = np.float32(1.0 / np.sqrt(HD))
    in_maps = []
    for c in range(N_CORES):
        b, g = divmod(c, 2)
        qr = slice(g * DV, g * DV + DV)
        kr = slice(D + g * DV, D + g * DV + DV)
        vr = slice(2 * D + g * DV, 2 * D + g * DV + DV)
        in_maps.append({
            "xT": np.ascontiguousarray(x[b].T),
            "wq": np.ascontiguousarray(qkv_w[qr].T * scale),
            "wk": np.ascontiguousarray(qkv_w[kr].T),
            "wv": np.ascontiguousarray(qkv_w[vr].T),
            "ow": np.ascontiguousarray(out_w[:, g * DV:g * DV + DV].T),
            "bq": (qkv_b[qr] * scale).reshape(DV, 1).astype(np.float32),
            "bk": qkv_b[kr].reshape(DV, 1).astype(np.float32),
            "kb": np.where(mask[b] != 0, 0.0, -1e30).astype(np.float32).reshape(S, 1),
        })
    return in_maps


def _host_gather(results, qkv_b, out_b, out_w):
    # constant bias: out_b + W_out @ v_bias (v bias commutes through attention)
    bias = out_b + out_w @ qkv_b[2 * D:3 * D]
    y = np.empty((B, S, D), dtype=np.float32)
    for b in range(B):
        y[b] = results[2 * b]["y"] + results[2 * b + 1]["y"] + bias[None, :]
    return y


def kernel(x, mask, qkv_w, qkv_b, out_w, out_b):
    import time
    from concourse.bass_utils import run_bass_kernel_spmd

    nc = _get_module()
    in_maps = _host_prep(x, mask, qkv_w, qkv_b, out_w)
    last = None
    for attempt in range(3):
        try:
            res = run_bass_kernel_spmd(nc, in_maps, core_ids=list(range(N_CORES)))
            return _host_gather(res.results, qkv_b, out_b, out_w)
        except Exception as e:  # rare transient device faults: retry after recovery
            last = e
            time.sleep(10 * (attempt + 1))
    raise last

